# revision 1
# baseline (speedup 1.0000x reference)
"""Asymmetric L1 loss on 8 Trainium2 NeuronCores.

reference: loss = sum(where(d<0, -penalty[j]*d, d)) / N  with d = computed - target.

Identity used: where(d<0, -p*d, d) = a_j*d + b_j*|d| with a=(1-p)/2, b=(1+p)/2.
So each core only needs per-column partial sums  B_j = sum(d),  A_j = sum(|d|)
over its shard; the penalty weighting + final scalar reduction happen on host
during the gather/unshard step (512 floats total).

Device kernel per core (data-parallel over rows; v5):
  - HWDGE f32 DMA, 2 MiB per transfer ([128, 4096] f32 tiles): measured
    407 GB/s on this setup (vs 353 at 4 MiB, 333 for SWDGE cast-DMA).
    Landing tiles are freed right after the subtract so DMA never waits on
    the long matmul chain.
  - DVE:  d = c - t into a separate bf16 tile (output cast is free; bf16
    keeps PE in 1-pass matmuls — fp32 matmuls are HI/LO 2-pass and were the
    95%-busy bottleneck in v1)
  - ACT:  a = |d| bf16
  - PE :  ones[128,1].T @ d-chunk  -> psum_B[1,512]  (accumulating matmuls)
          ones[128,1].T @ a-chunk  -> psum_A[1,512]
    (free index f of psum maps to column j = f % 32 — preserved because all
     chunk offsets are multiples of 32; PSUM accumulates fp32 so only the
     bf16 quantization of d enters the error, ~1e-5 on the final loss)
  - epilogue: strided tensor_reduce folds [1,512] -> [1,32] per accumulator,
    DMA out [1,64] = [B_j | A_j] in f32.
"""

import os
import sys

import numpy as np

if "/opt/trn_rl_repo" not in sys.path:
    sys.path.insert(0, "/opt/trn_rl_repo")

N_TOTAL = 4194304
M_COLS = 32
N_CORES = 8
N_PER_CORE = N_TOTAL // N_CORES          # 524288 rows per core
FLAT = N_PER_CORE * M_COLS               # 16777216 f32 per tensor per core
P = 128
PER_PART = FLAT // P                     # 131072 f32 per partition
TILE_F = 4096                            # f32 elems per partition per tile (2 MiB DMA)
MM_F = 512                               # matmul chunk (PSUM bank = 512 f32)

_cache = {}
_last_results = None


def schedule(n_tiles=PER_PART // TILE_F, tile_f=TILE_F, graded_tail=True, dma_f=None):
    if dma_f and dma_f != tile_f:
        # large-DMA schedule: dense dma_f-sized transfers, graded tail
        per_part = n_tiles * tile_f
        assert dma_f == 8192 and per_part % dma_f == 0
        tail_sizes = [4096, 4096, 2048, 2048, 1024, 1024, 512, 512, 512, 512]
        assert sum(tail_sizes) == 2 * dma_f
        pieces = []
        off = 0
        for _ in range(per_part // dma_f - 2):
            pieces.append((off, dma_f))
            off += dma_f
        for sz in tail_sizes:
            pieces.append((off, sz))
            off += sz
        return pieces, off
    pieces = []
    off = 0
    if graded_tail and n_tiles > 2:
        tail_sizes = [2048, 2048, 1024, 1024, 1024, 512, 512]
        assert sum(tail_sizes) == 2 * tile_f
        for _ in range(n_tiles - 2):
            pieces.append((off, tile_f))
            off += tile_f
        for sz in tail_sizes:
            pieces.append((off, sz))
            off += sz
    else:
        for _ in range(n_tiles):
            pieces.append((off, tile_f))
            off += tile_f
    return pieces, off


def host_pretile(shard_2d, pieces):
    """Reorder a [P, per_part] shard so each piece's [P, sz] tile is one
    contiguous DRAM block (partition-major within the block)."""
    blocks = [np.ascontiguousarray(shard_2d[:, off:off + sz]).reshape(-1)
              for off, sz in pieces]
    return np.concatenate(blocks)


def build(n_tiles=PER_PART // TILE_F, per_part=PER_PART, tile_f=TILE_F,
          graded_tail=True, warmup=False, dual_ring=False, contig=False,
          dma_f=None):
    from concourse import bacc, mybir, tile

    assert n_tiles * tile_f == per_part
    # schedule: big tiles, with the final two tiles graded down to small
    # pieces so the post-last-DMA chain (sub -> abs -> matmuls -> epilogue)
    # drains fast
    pieces, off = schedule(n_tiles, tile_f, graded_tail, dma_f=dma_f)
    assert off == per_part
    land_f = max(sz for _, sz in pieces)     # landing-tile width (DMA size)

    nc = bacc.Bacc(None, target_bir_lowering=False)
    f32 = mybir.dt.float32
    bf16 = mybir.dt.bfloat16
    if contig:
        # pre-tiled layout: each piece's [P, sz] tile is one dense DRAM block
        c_dram = nc.declare_dram_parameter("computed", [P * per_part], f32, isOutput=False)
        t_dram = nc.declare_dram_parameter("target", [P * per_part], f32, isOutput=False)

        def src(dram, off, sz):
            base = off * P
            return dram[base:base + P * sz].rearrange("(p f) -> p f", p=P)
    else:
        c_dram = nc.declare_dram_parameter("computed", [P, per_part], f32, isOutput=False)
        t_dram = nc.declare_dram_parameter("target", [P, per_part], f32, isOutput=False)

        def src(dram, off, sz):
            return dram[:, off:off + sz]
    out_dram = nc.declare_dram_parameter("out", [1, 64], f32, isOutput=True)

    cbufs = 4 if land_f <= 4096 else 2       # SBUF: keep pools under ~176KB/partition
    dbufs = 2 if land_f <= 4096 else 3
    with tile.TileContext(nc) as tc:
        with (
            tc.tile_pool(name="cpool", bufs=cbufs) as cpool,
            tc.tile_pool(name="tpool", bufs=cbufs) as tpool,
            tc.tile_pool(name="dpool", bufs=dbufs) as dpool,
            tc.tile_pool(name="apool", bufs=dbufs) as apool,
            tc.tile_pool(name="kpool", bufs=1) as kpool,
            tc.tile_pool(name="fpool", bufs=1) as fpool,
            tc.tile_pool(name="psum", bufs=1, space="PSUM") as psum_pool,
        ):
            ones = kpool.tile([P, 1], bf16)
            nc.vector.memset(ones[:], 1.0)
            if warmup:
                # tiny DMA to wake the HWDGE ring(s)/SDMA engines before the
                # first real transfer
                wu = kpool.tile([1, 16], f32)
                nc.sync.dma_start(out=wu[:], in_=src(c_dram, 0, 16)[0:1, :])
                if dual_ring:
                    wu2 = kpool.tile([1, 16], f32)
                    nc.scalar.dma_start(out=wu2[:], in_=src(t_dram, 0, 16)[0:1, :])
            psum_b_t = psum_pool.tile([1, MM_F], f32, tag="pb", name="psum_b")
            psum_a_t = psum_pool.tile([1, MM_F], f32, tag="pa", name="psum_a")
            psum_b = psum_b_t[:, :]
            psum_a = psum_a_t[:, :]

            for pi, (off, sz) in enumerate(pieces):
                c = cpool.tile([P, land_f], f32, tag="c")
                t = tpool.tile([P, land_f], f32, tag="t")
                nc.sync.dma_start(out=c[:, 0:sz], in_=src(c_dram, off, sz))
                t_dma = nc.scalar if dual_ring else nc.sync
                t_dma.dma_start(out=t[:, 0:sz], in_=src(t_dram, off, sz))
                last_piece = pi == len(pieces) - 1
                for so in range(0, sz, tile_f):
                    cs = min(tile_f, sz - so)
                    d = dpool.tile([P, tile_f], bf16, tag="d")
                    a = apool.tile([P, tile_f], bf16, tag="a")
                    nc.vector.tensor_sub(
                        out=d[:, 0:cs], in0=c[:, so:so + cs], in1=t[:, so:so + cs]
                    )
                    nc.scalar.activation(
                        out=a[:, 0:cs], in_=d[:, 0:cs],
                        func=mybir.ActivationFunctionType.Abs,
                    )
                    for m in range(cs // MM_F):
                        first = pi == 0 and so == 0 and m == 0
                        last = last_piece and so + tile_f >= sz and m == cs // MM_F - 1
                        nc.tensor.matmul(
                            psum_b, ones[:], d[:, m * MM_F:(m + 1) * MM_F],
                            start=first, stop=last,
                        )
                        nc.tensor.matmul(
                            psum_a, ones[:], a[:, m * MM_F:(m + 1) * MM_F],
                            start=first, stop=last,
                        )

            res = fpool.tile([1, 64], f32)
            nc.vector.tensor_reduce(
                out=res[:, 0:32],
                in_=psum_b.rearrange("p (r j) -> p j r", j=M_COLS),
                axis=mybir.AxisListType.X,
                op=mybir.AluOpType.add,
            )
            nc.vector.tensor_reduce(
                out=res[:, 32:64],
                in_=psum_a.rearrange("p (r j) -> p j r", j=M_COLS),
                axis=mybir.AxisListType.X,
                op=mybir.AluOpType.add,
            )
            nc.sync.dma_start(out=out_dram[:], in_=res[:])

    nc.compile()
    return nc


def kernel(computed, target, penalty):
    global _last_results
    from concourse.bass_utils import run_bass_kernel_spmd

    if "nc" not in _cache:
        _cache["nc"] = build(contig=True)
    nc = _cache["nc"]
    pieces, _ = schedule()

    computed = np.ascontiguousarray(computed, dtype=np.float32)
    target = np.ascontiguousarray(target, dtype=np.float32)
    in_maps = []
    for i in range(N_CORES):
        sl = slice(i * N_PER_CORE, (i + 1) * N_PER_CORE)
        in_maps.append(
            {
                "computed": host_pretile(computed[sl].reshape(P, PER_PART), pieces),
                "target": host_pretile(target[sl].reshape(P, PER_PART), pieces),
            }
        )

    trace = bool(os.environ.get("KERNEL_TRACE"))
    res = run_bass_kernel_spmd(nc, in_maps, core_ids=list(range(N_CORES)), trace=trace)
    _last_results = res

    B = np.zeros(M_COLS, np.float64)
    A = np.zeros(M_COLS, np.float64)
    for r in res.results:
        out = np.asarray(r["out"]).reshape(64).astype(np.float64)
        B += out[:32]
        A += out[32:]
    p = np.asarray(penalty, dtype=np.float64)
    aw = (1.0 - p) / 2.0
    bw = (1.0 + p) / 2.0
    total = float(aw @ B + bw @ A)
    return np.float32(total / N_TOTAL)

